# revision 1
# baseline (speedup 1.0000x reference)
"""GCN (4-layer, PyG GCNConv semantics) on 8 Trainium2 NeuronCores — v2.

Sharding: nodes partitioned into 8 contiguous blocks (graph-parallel); edges
assigned to the core owning their dst. Per layer: local dense matmul
g=(h*dis)@W written in duplicated-fp16 format (512B rows gather ~4x faster
than 256B), 4 chunked AllGathers build segment tables of <=25600 rows (int16
dma_gather index limit), then per-core batched dma_gather of src rows +
segment-sum via one-hot selection matmuls accumulating in PSUM. The chunked
AG lets gathers on segment s start as soon as AG s lands.

Self-contained: hardcodes N=100000, E=1600000, B=64, D=128.
"""

import numpy as np

import concourse.bacc as bacc
import concourse.bass as bass
import concourse.mybir as mybir
import concourse.tile as tile

N = 100000
E = 1600000
B = 64
D = 128
NC = 8
NPC = N // NC            # real nodes per core (12500)
NT = 100                 # padded node tiles per core (12800 slots; 98 real)
NTR = 98                 # real tiles
NB = NT * 128            # padded node slots per core (12800)
NSEG = 2
SEGT = NT // NSEG        # tiles per AG segment (50)
SEGR = SEGT * 128        # node rows per AG segment per core (6400)
TBLR = NC * SEGR         # node rows per segment table (51200); pair rows 25600 < 32767
TB = 8                   # dst tiles per gather batch
NQ = 4                   # SWDGE queues
GMAX = 8                 # max chunks (of 128 rows) per dma_gather (ring cap 1024)

F32 = mybir.dt.float32
I32 = mybir.dt.int32
I16 = mybir.dt.int16
GDT = mybir.dt.float16
GNP = np.float16

_CACHE = {}


def _preprocess(x, edge_index, batch, question_embedding,
                w0, b0, w1, b1, w2, b2, w3, b3,
                fc0_w, fc0_b, fc1_w, fc1_b, fc2_w, fc2_b):
    src = np.asarray(edge_index[0], dtype=np.int64)
    dst = np.asarray(edge_index[1], dtype=np.int64)
    x = np.asarray(x, dtype=np.float32)
    batch = np.asarray(batch, dtype=np.int64)

    deg = (np.bincount(dst, minlength=N) + 1).astype(np.float32)
    dis = deg ** -0.5

    core = dst // NPC
    rem = dst - core * NPC
    tile_of = rem >> 7                      # dst tile [0,98)
    slot = rem & 127
    score = src // NPC
    srow = src - score * NPC
    seg = srow // SEGR                      # src segment [0,2)
    within = score * SEGR + (srow - seg * SEGR)    # node row in segment table
    pair = within >> 1                             # pair row (512B granularity)
    par = within & 1                               # which 128-col half

    # order edges by (core, batch, seg, parity, tile); SPMD needs ONE schedule,
    # so the chunk layout per (batch, seg, par, tile) bucket is MAX over cores.
    batch_of = tile_of // TB
    key = (((core * 16 + batch_of) * NSEG + seg) * 2 + par) * NT + tile_of
    order = np.argsort(key, kind="stable")
    idx_s = pair[order].astype(np.int16)
    slot_s = slot[order].astype(np.float32)

    nbatch = (NTR + TB - 1) // TB  # 13
    NKEY = NC * 16 * NSEG * 2 * NT
    cnt = np.bincount(key[order], minlength=NKEY)
    starts_all = np.zeros(NKEY + 1, dtype=np.int64)
    starts_all[1:] = np.cumsum(cnt)
    cnt = cnt.reshape(NC, 16, NSEG, 2, NT)
    nch_common = np.ceil(cnt / 128).astype(np.int64).max(axis=0)  # [16,NSEG,2,NT]

    # common schedule: chunk = (tile, seg, par)
    chunks = []
    batches = []         # per batch: tiles list + per-seg (chunk0, nchunks)
    bucket_c0 = {}       # (bi, s, p, t) -> first chunk index
    for bi in range(nbatch):
        tiles = list(range(bi * TB, min((bi + 1) * TB, NTR)))
        seg_runs = {}
        for s in range(NSEG):
            c0 = len(chunks)
            for t in tiles:
                for p in range(2):
                    bucket_c0[(bi, s, p, t)] = len(chunks)
                    for _ in range(int(nch_common[bi, s, p, t])):
                        chunks.append((t, s, p))
            seg_runs[s] = (c0, len(chunks) - c0)
        batches.append({"tiles": tiles, "seg_runs": seg_runs})
    CT = len(chunks)
    by_tile = [[] for _ in range(NTR)]
    for k, (t, s, p) in enumerate(chunks):
        by_tile[t].append((k, s, p))
    meta = {"chunks": chunks, "batches": batches, "by_tile": by_tile, "CT": CT}

    # per-core idx/dstf fills into the common chunk slots
    idx16_list, dstf_list = [], []
    for c in range(NC):
        idx16 = np.zeros((16, CT * 8), dtype=np.int16)
        dstf = np.full((128, CT), -1.0, dtype=np.float16)
        for bi in range(nbatch):
            for s in range(NSEG):
                for t in range(bi * TB, min((bi + 1) * TB, NTR)):
                    for p in range(2):
                        kbase = bucket_c0[(bi, s, p, t)]
                        kk = (((c * 16 + bi) * NSEG + s) * 2 + p) * NT + t
                        e0, e1 = starts_all[kk], starts_all[kk + 1]
                        n = int(e1 - e0)
                        if n == 0:
                            continue
                        cap = int(nch_common[bi, s, p, t]) * 128
                        assert n <= cap
                        buf_i = np.zeros(cap, dtype=np.int16)
                        buf_i[:n] = idx_s[e0:e1]
                        buf_s = np.full(cap, -1.0, dtype=np.float16)
                        buf_s[:n] = slot_s[e0:e1]
                        nch = cap // 128
                        idx16[:, kbase * 8:(kbase + nch) * 8] = \
                            buf_i.reshape(nch * 8, 16).T
                        dstf[:, kbase:kbase + nch] = buf_s.reshape(nch, 128).T
        idx16_list.append(np.tile(idx16, (8, 1)))
        dstf_list.append(dstf)

    # per-core padded node data
    xds = x * dis[:, None]
    dis_pad = np.zeros((NC, NB), dtype=np.float32)
    xdT = np.zeros((NC, 128, NB), dtype=np.float32)
    qidx16 = np.zeros((NC, 128, NB // 16), dtype=np.int16)
    for c in range(NC):
        blk = slice(c * NPC, (c + 1) * NPC)
        dis_pad[c, :NPC] = dis[blk]
        xdT[c, :, :NPC] = xds[blk].T
        bp = np.zeros(NB, dtype=np.int16)
        bp[:NPC] = batch[blk].astype(np.int16)
        q16 = bp.reshape(NB // 16, 16).T     # [16, NB/16]
        qidx16[c] = np.tile(q16, (8, 1))
    discolT = dis_pad.reshape(NC, NT, 128).transpose(0, 2, 1).copy()  # [NC,128,NT]

    # weights (replicated)
    w_all = np.concatenate([np.asarray(w, np.float32) for w in (w0, w1, w2, w3)], axis=0)
    brep1 = np.concatenate(
        [np.tile(np.asarray(b, np.float32)[None, :] + 1.0, (128, 1)) for b in (b0, b1, b2, b3)],
        axis=0,
    )
    fc0_w = np.asarray(fc0_w, np.float32)
    fc0w_packed = np.concatenate([fc0_w[j * 128:(j + 1) * 128, :] for j in range(6)], axis=1)
    qT = np.asarray(question_embedding, np.float32).T
    qembT_packed = np.concatenate([qT[j * 128:(j + 1) * 128, :] for j in range(6)], axis=1)
    fc0_b1_rep = np.tile(np.asarray(fc0_b, np.float32)[None, :] + 1.0, (128, 1))
    fc1a = np.ascontiguousarray(np.asarray(fc1_w, np.float32)[:128, :])
    fc1b = np.ascontiguousarray(np.asarray(fc1_w, np.float32)[128:, :])
    fc1_b1_rep = np.tile(np.asarray(fc1_b, np.float32)[None, :] + 1.0, (128, 1))
    fc2_w = np.asarray(fc2_w, np.float32)
    fc2_b_rep = np.tile(np.asarray(fc2_b, np.float32)[None, :], (128, 1))
    iota_big = np.tile(np.arange(128, dtype=np.float16), GMAX)[None, :].repeat(128, 0)  # [128, 8*128]
    ident = np.eye(128, dtype=np.float32)
    ident_g = np.eye(128, dtype=GNP)

    shared = dict(
        w_all=w_all, brep1=brep1, fc0w_packed=fc0w_packed, qembT_packed=qembT_packed,
        fc0_b1_rep=fc0_b1_rep, fc1a=fc1a, fc1b=fc1b, fc1_b1_rep=fc1_b1_rep,
        fc2_w=fc2_w, fc2_b_rep=fc2_b_rep, iota_big=iota_big, ident=ident, ident_g=ident_g,
    )
    in_maps = []
    for c in range(NC):
        m = dict(shared)
        m.update(
            xdT=np.ascontiguousarray(xdT[c]),
            discolT=np.ascontiguousarray(discolT[c]),
            qidx16=np.ascontiguousarray(qidx16[c]),
            idx_all=np.ascontiguousarray(idx16_list[c]),
            dstf_all=np.ascontiguousarray(dstf_list[c]),
        )
        in_maps.append(m)
    return in_maps, meta, CT


def _elu_chain(nc, sbuf, psum_or_sbuf, dis_scale, brep1_t, neg1, extra_add=None):
    t = sbuf.tile([128, D], F32, tag="elu_t")
    nc.scalar.activation(t[:], psum_or_sbuf, mybir.ActivationFunctionType.Copy, scale=dis_scale)
    u1 = sbuf.tile([128, D], F32, tag="elu_u1")
    nc.vector.tensor_add(u1[:], t[:], brep1_t)
    m = sbuf.tile([128, D], F32, tag="elu_m")
    nc.vector.tensor_scalar(m[:], u1[:], 1.0, None, mybir.AluOpType.min)
    e = sbuf.tile([128, D], F32, tag="elu_e")
    nc.scalar.activation(e[:], m[:], mybir.ActivationFunctionType.Exp, bias=neg1)
    v = sbuf.tile([128, D], F32, tag="elu_v")
    nc.vector.tensor_tensor(v[:], u1[:], e[:], op=mybir.AluOpType.max)
    h = sbuf.tile([128, D], F32, tag="elu_h")
    nc.scalar.activation(h[:], v[:], mybir.ActivationFunctionType.Copy, bias=-1.0)
    if extra_add is not None:
        h2 = sbuf.tile([128, D], F32, tag="elu_h2")
        nc.vector.tensor_add(h2[:], h[:], extra_add)
        return h2
    return h


def _build(meta0, CTmax):
    import os
    layer_reps = int(os.environ.get("LAYER_REPS", 1))
    head_reps = int(os.environ.get("HEAD_REPS", 1))
    ag0 = bool(int(os.environ.get("AG0", "0")))      # skip AllGathers
    small_out = bool(int(os.environ.get("SMALL_OUT", "0")))  # tiny output (timing)
    gx2 = bool(int(os.environ.get("GX2", "0")))      # issue each gather twice
    sx2 = bool(int(os.environ.get("SX2", "0")))      # build each sel twice
    g256 = bool(int(os.environ.get("G256", "0")))    # timing-only: 256B gather rows
    gsz = int(os.environ.get("GSZ", GMAX))           # chunks per gather (<=8)
    gbufs = int(os.environ.get("GBUFS", 20))
    nc = bacc.Bacc("TRN2", target_bir_lowering=False, debug=False, num_devices=NC,
                   num_swdge_queues=NQ)

    # per-core inputs
    xdT = nc.dram_tensor("xdT", [128, NB], F32, kind="ExternalInput")
    discolT = nc.dram_tensor("discolT", [128, NT], F32, kind="ExternalInput")
    qidx16 = nc.dram_tensor("qidx16", [128, NB // 16], I16, kind="ExternalInput")
    idx_all = nc.dram_tensor("idx_all", [128, CTmax * 8], I16, kind="ExternalInput")
    dstf_all = nc.dram_tensor("dstf_all", [128, CTmax], GDT, kind="ExternalInput")
    # replicated inputs
    w_all = nc.dram_tensor("w_all", [512, D], F32, kind="ExternalInput")
    brep1 = nc.dram_tensor("brep1", [512, D], F32, kind="ExternalInput")
    fc0w_packed = nc.dram_tensor("fc0w_packed", [128, 768], F32, kind="ExternalInput")
    qembT_packed = nc.dram_tensor("qembT_packed", [128, 384], F32, kind="ExternalInput")
    fc0_b1_rep = nc.dram_tensor("fc0_b1_rep", [128, D], F32, kind="ExternalInput")
    fc1a = nc.dram_tensor("fc1a", [128, D], F32, kind="ExternalInput")
    fc1b = nc.dram_tensor("fc1b", [128, D], F32, kind="ExternalInput")
    fc1_b1_rep = nc.dram_tensor("fc1_b1_rep", [128, D], F32, kind="ExternalInput")
    fc2_w = nc.dram_tensor("fc2_w", [128, D], F32, kind="ExternalInput")
    fc2_b_rep = nc.dram_tensor("fc2_b_rep", [128, D], F32, kind="ExternalInput")
    iota_in = nc.dram_tensor("iota_big", [128, GMAX * 128], GDT, kind="ExternalInput")
    ident_in = nc.dram_tensor("ident", [128, 128], F32, kind="ExternalInput")
    ident_g_in = nc.dram_tensor("ident_g", [128, 128], GDT, kind="ExternalInput")

    out = nc.dram_tensor("out", [128 if small_out else NB, D], F32,
                         kind="ExternalOutput")

    NL = 4 * layer_reps
    with tile.TileContext(nc) as tc:
        with (
            tc.tile_pool(name="const", bufs=1) as cpool,
            tc.tile_pool(name="wpool", bufs=2) as wpool,
            tc.tile_pool(name="dense", bufs=4) as dense,
            tc.tile_pool(name="gmeta", bufs=3) as gmeta,
            tc.tile_pool(name="gather", bufs=gbufs) as gpool,
            tc.tile_pool(name="sel", bufs=6) as selpool,
            tc.tile_pool(name="fin", bufs=3) as fin,
            tc.tile_pool(name="qpool", bufs=3) as qpool,
            tc.tile_pool(name="psum", bufs=2, space="PSUM") as psum,
            tc.tile_pool(name="apsum", bufs=3, space="PSUM") as apsum,
            tc.tile_pool(name="dram", bufs=1, space="DRAM") as dram,
        ):
            ag_in = [dram.tile([SEGR, D], GDT, tag=f"ag_in{s}", name=f"ag_in{s}")
                     for s in range(NSEG)]
            g_segs = [[dram.tile([TBLR, D], GDT, addr_space="Shared",
                                 tag=f"g_seg{l}_{s}", name=f"g_seg{l}_{s}")
                       for s in range(NSEG)] for l in range(NL)]
            hdT_a = dram.tile([128, NB], F32, tag="hdT_a")
            hdT_b = dram.tile([128, NB], F32, tag="hdT_b")
            h2_buf = dram.tile([NB, D], F32, tag="h2_buf")
            qq_dram = dram.tile([B, D], F32, tag="qq_dram")

            # constants
            iota_t = cpool.tile([128, GMAX * 128], GDT)
            nc.sync.dma_start(out=iota_t[:], in_=iota_in[:, :])
            iota3 = iota_t[:].rearrange("p (k f) -> p k f", k=GMAX)
            ident_t = cpool.tile([128, 128], F32)
            nc.sync.dma_start(out=ident_t[:], in_=ident_in[:, :])
            identg_t = cpool.tile([128, 128], GDT)
            nc.sync.dma_start(out=identg_t[:], in_=ident_g_in[:, :])
            neg1 = cpool.tile([128, 1], F32)
            nc.vector.memset(neg1[:], -1.0)
            dis_t = cpool.tile([128, NT], F32)
            nc.sync.dma_start(out=dis_t[:], in_=discolT[:, :])
            qidx_t = cpool.tile([128, NB // 16], I16)
            nc.sync.dma_start(out=qidx_t[:], in_=qidx16[:, :])

            # ---------------- question head (replicated) ----------------
            qembT_t = cpool.tile([128, 384], F32)
            nc.sync.dma_start(out=qembT_t[:], in_=qembT_packed[:, :])
            fc0w_t = cpool.tile([128, 768], F32)
            nc.sync.dma_start(out=fc0w_t[:], in_=fc0w_packed[:, :])
            fc0b1_t = cpool.tile([128, D], F32)
            nc.sync.dma_start(out=fc0b1_t[:], in_=fc0_b1_rep[:, :])
            fc1b_t = cpool.tile([128, D], F32)
            nc.sync.dma_start(out=fc1b_t[:], in_=fc1b[:, :])
            fc1b1_t = cpool.tile([128, D], F32)
            nc.sync.dma_start(out=fc1b1_t[:], in_=fc1_b1_rep[:, :])

            q0_ps = psum.tile([64, D], F32, tag="dmm")
            for j in range(6):
                nc.tensor.matmul(
                    q0_ps[:], qembT_t[:, j * 64:(j + 1) * 64], fc0w_t[:, j * 128:(j + 1) * 128],
                    start=(j == 0), stop=(j == 5),
                )
            qu1 = cpool.tile([64, D], F32)
            nc.vector.tensor_add(qu1[:], q0_ps[:], fc0b1_t[:64, :])
            qm = cpool.tile([64, D], F32)
            nc.vector.tensor_scalar(qm[:], qu1[:], 1.0, None, mybir.AluOpType.min)
            qe = cpool.tile([64, D], F32)
            nc.scalar.activation(qe[:], qm[:], mybir.ActivationFunctionType.Exp, bias=neg1[:64, :1])
            qv = cpool.tile([64, D], F32)
            nc.vector.tensor_tensor(qv[:], qu1[:], qe[:], op=mybir.AluOpType.max)
            q_t = cpool.tile([64, D], F32)
            nc.scalar.activation(q_t[:], qv[:], mybir.ActivationFunctionType.Copy, bias=-1.0)
            qT_ps = psum.tile([128, 64], F32, tag="tp")
            nc.tensor.transpose(qT_ps[:], q_t[:], ident_t[:64, :64])
            qT_t = cpool.tile([128, 64], F32)
            nc.vector.tensor_copy(qT_t[:], qT_ps[:])
            qq_ps = psum.tile([64, D], F32, tag="dmm")
            nc.tensor.matmul(qq_ps[:], qT_t[:], fc1b_t[:], start=True, stop=True)
            qq_t = cpool.tile([64, D], F32)
            nc.vector.tensor_add(qq_t[:], qq_ps[:], fc1b1_t[:64, :])
            nc.sync.dma_start(out=qq_dram[:, :], in_=qq_t[:])

            # ---------------- GCN layers ----------------
            dense_src = [xdT, hdT_a, hdT_b, hdT_a] * layer_reps
            agg_dstT = [hdT_a, hdT_b, hdT_a, hdT_b] * layer_reps
            for rep in range(layer_reps - 1):
                dense_src[4 * (rep + 1)] = hdT_b
            gq = [0]  # round-robin queue counter
            NCHB_MAX = max(
                (b["seg_runs"][NSEG - 1][0] + b["seg_runs"][NSEG - 1][1]
                 - b["seg_runs"][0][0])
                for b in meta0["batches"])

            for glayer in range(NL):
                layer = glayer % 4
                w_t = wpool.tile([128, D], F32, tag="w")
                nc.sync.dma_start(out=w_t[:], in_=w_all[layer * 128:(layer + 1) * 128, :])
                b1_t = wpool.tile([128, D], F32, tag="b")
                nc.sync.dma_start(out=b1_t[:], in_=brep1[layer * 128:(layer + 1) * 128, :])

                # dense: g = (h*dis) @ w -> duplicated fp16 rows in ag_in segs
                hsrc = dense_src[glayer]
                for t in range(NTR):
                    hdT_t = dense.tile([128, 128], F32, tag="hdT_in")
                    nc.sync.dma_start(out=hdT_t[:], in_=hsrc[:, t * 128:(t + 1) * 128])
                    g_ps = psum.tile([128, D], F32, tag="dmm")
                    nc.tensor.matmul(g_ps[:], hdT_t[:], w_t[:], start=True, stop=True)
                    g_sb = dense.tile([128, D], GDT, tag="g_out")
                    nc.scalar.copy(g_sb[:], g_ps[:])
                    s, tr = t // SEGT, t % SEGT
                    nc.sync.dma_start(
                        out=ag_in[s][tr * 128:(tr + 1) * 128, :], in_=g_sb[:])
                # zero-fill padding tiles once (layer 0 only; pads stay zero after)
                if glayer == 0:
                    zpad = dense.tile([128, D], GDT, tag="zpad")
                    nc.vector.memset(zpad[:], 0.0)
                    for t in range(NTR, NT):
                        s, tr = t // SEGT, t % SEGT
                        nc.sync.dma_start(
                            out=ag_in[s][tr * 128:(tr + 1) * 128, :], in_=zpad[:])

                if not ag0:
                    for s in range(NSEG):
                        nc.gpsimd.collective_compute(
                            "AllGather",
                            mybir.AluOpType.bypass,
                            replica_groups=[list(range(NC))],
                            ins=[ag_in[s].opt()],
                            outs=[g_segs[glayer][s].opt()],
                        )

                # aggregation per batch
                hdst = agg_dstT[glayer]
                # map chunk -> (gather tile, slot) built on the fly
                chunk_home = {}
                for binfo in meta0["batches"]:
                    # stage this batch's int16 indices in SBUF, then gather
                    cb0 = binfo["seg_runs"][0][0]
                    cb1 = binfo["seg_runs"][NSEG - 1][0] + binfo["seg_runs"][NSEG - 1][1]
                    nchb = cb1 - cb0
                    idx_t = gmeta.tile([128, NCHB_MAX * 8], I16, tag="idx")
                    nc.sync.dma_start(
                        out=idx_t[:, 0:nchb * 8], in_=idx_all[:, cb0 * 8:cb1 * 8])
                    for s in range(NSEG):
                        c0, nch = binfo["seg_runs"][s]
                        off = 0
                        while off < nch:
                            g_n = min(gsz, nch - off)
                            gw = D if g256 else 2 * D
                            gt = gpool.tile([128, GMAX, gw], GDT, tag="gbuf")
                            k0 = c0 + off
                            pair_view = g_segs[glayer][s][:, :].rearrange(
                                "(a two) f -> a (two f)", two=2)
                            for _rep in range(2 if gx2 else 1):
                                nc.gpsimd.dma_gather(
                                    gt[:, 0:g_n, :],
                                    pair_view[:, 0:gw] if g256 else pair_view,
                                    idx_t[:, (k0 - cb0) * 8:(k0 - cb0 + g_n) * 8],
                                    g_n * 128, g_n * 128, gw,
                                    elem_step=(2 * D) if g256 else None,
                                    queue_num=gq[0] % NQ,
                                )
                                gq[0] += 1
                            for j in range(g_n):
                                chunk_home[k0 + j] = (gt, j)
                            off += g_n

                    # process tiles of this batch
                    for t in binfo["tiles"]:
                        tl = meta0["by_tile"][t]
                        agg_ps = apsum.tile([128, D], F32, tag="agg")
                        first = True
                        # group tile-chunks into consecutive runs (sel build
                        # spans parities; matmul picks the half per chunk)
                        i = 0
                        while i < len(tl):
                            k0 = tl[i][0]
                            j = i
                            while j + 1 < len(tl) and tl[j + 1][0] == tl[j][0] + 1 \
                                    and j + 1 - i < GMAX:
                                j += 1
                            nsel = j - i + 1
                            dstf_t = gmeta.tile([128, GMAX], GDT, tag="dstf")
                            nc.sync.dma_start(
                                out=dstf_t[:, 0:nsel], in_=dstf_all[:, k0:k0 + nsel])
                            sel = selpool.tile([128, GMAX, 128], GDT, tag="sel")
                            for _rep in range(2 if sx2 else 1):
                                nc.vector.tensor_tensor(
                                    sel[:, 0:nsel, :],
                                    dstf_t[:, 0:nsel].unsqueeze(2).broadcast_to([128, nsel, 128]),
                                    iota3[:, 0:nsel, :],
                                    op=mybir.AluOpType.is_equal,
                                )
                            for q in range(nsel):
                                k = k0 + q
                                gt, gslot = chunk_home[k]
                                p = 0 if g256 else tl[i + q][2]
                                nc.tensor.matmul(
                                    agg_ps[:], sel[:, q, :],
                                    gt[:, gslot, p * D:(p + 1) * D],
                                    start=first, stop=False,
                                )
                                first = False
                            i = j + 1
                        # self-loop
                        gself = fin.tile([128, D], GDT, tag="gself")
                        s, tr = t // SEGT, t % SEGT
                        nc.sync.dma_start(
                            out=gself[:], in_=ag_in[s][tr * 128:(tr + 1) * 128, 0:D])
                        nc.tensor.matmul(agg_ps[:], identg_t[:], gself[:],
                                         start=first, stop=True)

                        extra = None
                        if layer == 3:
                            extra_t = fin.tile([128, D], F32, tag="h2in")
                            nc.sync.dma_start(out=extra_t[:], in_=h2_buf[t * 128:(t + 1) * 128, :])
                            extra = extra_t[:]
                        h_t = _elu_chain(nc, fin, agg_ps[:], dis_t[:, t:t + 1],
                                         b1_t[:], neg1[:, :1], extra_add=extra)

                        if layer == 1:
                            nc.sync.dma_start(out=h2_buf[t * 128:(t + 1) * 128, :], in_=h_t[:])
                        if layer < 3:
                            hd = fin.tile([128, D], F32, tag="hd")
                            nc.vector.tensor_scalar(
                                hd[:], h_t[:], dis_t[:, t:t + 1], None, mybir.AluOpType.mult)
                            tp_ps = psum.tile([128, 128], F32, tag="tp")
                            nc.tensor.transpose(tp_ps[:], hd[:], ident_t[:])
                        else:
                            tp_ps = psum.tile([128, 128], F32, tag="tp")
                            nc.tensor.transpose(tp_ps[:], h_t[:], ident_t[:])
                        hdT_o = fin.tile([128, 128], F32, tag="hdT_out")
                        nc.scalar.copy(hdT_o[:], tp_ps[:])
                        nc.sync.dma_start(out=hdst[:, t * 128:(t + 1) * 128], in_=hdT_o[:])

            # ---------------- MLP head ----------------
            fc1a_t = cpool.tile([128, D], F32)
            nc.sync.dma_start(out=fc1a_t[:], in_=fc1a[:, :])
            fc2w_t = cpool.tile([128, D], F32)
            nc.sync.dma_start(out=fc2w_t[:], in_=fc2_w[:, :])
            fc2b_t = cpool.tile([128, D], F32)
            nc.sync.dma_start(out=fc2b_t[:], in_=fc2_b_rep[:, :])

            h4T = agg_dstT[3]
            qq_home = {}
            for hrep in range(head_reps):
              qq_home = {}
              for bt in range(0, NTR, GMAX):
                  g_n = min(GMAX, NTR - bt)
                  qg = qpool.tile([128, GMAX, D], F32, tag="qqg")
                  nc.gpsimd.dma_gather(
                      qg[:, 0:g_n, :],
                      qq_dram[:, :],
                      qidx_t[:, bt * 8:(bt + g_n) * 8],
                      g_n * 128, g_n * 128, D,
                      queue_num=gq[0] % NQ,
                  )
                  gq[0] += 1
                  for j in range(g_n):
                      qq_home[bt + j] = (qg, j)
              for t in range(NTR):
                  h4T_t = dense.tile([128, 128], F32, tag="hdT_in")
                  nc.sync.dma_start(out=h4T_t[:], in_=h4T[:, t * 128:(t + 1) * 128])
                  mm1_ps = psum.tile([128, D], F32, tag="dmm")
                  nc.tensor.matmul(mm1_ps[:], h4T_t[:], fc1a_t[:], start=True, stop=True)
                  qg, qslot = qq_home[t]
                  u1 = fin.tile([128, D], F32, tag="elu_u1")
                  nc.vector.tensor_add(u1[:], mm1_ps[:], qg[:, qslot, :])
                  m = fin.tile([128, D], F32, tag="elu_m")
                  nc.vector.tensor_scalar(m[:], u1[:], 1.0, None, mybir.AluOpType.min)
                  e = fin.tile([128, D], F32, tag="elu_e")
                  nc.scalar.activation(e[:], m[:], mybir.ActivationFunctionType.Exp, bias=neg1[:, :1])
                  v = fin.tile([128, D], F32, tag="elu_v")
                  nc.vector.tensor_tensor(v[:], u1[:], e[:], op=mybir.AluOpType.max)
                  o1 = fin.tile([128, D], F32, tag="elu_h")
                  nc.scalar.activation(o1[:], v[:], mybir.ActivationFunctionType.Copy, bias=-1.0)
                  tp_ps = psum.tile([128, 128], F32, tag="tp")
                  nc.tensor.transpose(tp_ps[:], o1[:], ident_t[:])
                  o1T = fin.tile([128, 128], F32, tag="hdT_out")
                  nc.scalar.copy(o1T[:], tp_ps[:])
                  mm2_ps = psum.tile([128, D], F32, tag="dmm")
                  nc.tensor.matmul(mm2_ps[:], o1T[:], fc2w_t[:], start=True, stop=True)
                  o2 = fin.tile([128, D], F32, tag="out2")
                  nc.vector.tensor_add(o2[:], mm2_ps[:], fc2b_t[:])
                  if small_out:
                      if t == 0:
                          nc.sync.dma_start(out=out[0:128, :], in_=o2[:])
                  else:
                      nc.sync.dma_start(out=out[t * 128:(t + 1) * 128, :], in_=o2[:])

    nc.compile()
    return nc


def _get_compiled(inputs):
    in_maps, meta, CTmax = _preprocess(**inputs)
    import os
    key = ("v3", os.environ.get("LAYER_REPS", ""), os.environ.get("HEAD_REPS", ""),
           os.environ.get("AG0", ""), os.environ.get("GX2", ""), os.environ.get("SX2", ""),
           os.environ.get("SMALL_OUT", ""), os.environ.get("G256", ""),
           os.environ.get("GSZ", ""), os.environ.get("GBUFS", ""), CTmax)
    if key not in _CACHE:
        _CACHE[key] = _build(meta, CTmax)
    return _CACHE[key], in_maps


def kernel(**inputs) -> np.ndarray:
    from concourse.bass_utils import run_bass_kernel_spmd

    nc, in_maps = _get_compiled(inputs)
    res = run_bass_kernel_spmd(nc, in_maps, core_ids=list(range(NC)))
    out = np.concatenate([res.results[c]["out"][:NPC] for c in range(NC)], axis=0)
    return out.astype(np.float32)


if __name__ == "__main__":
    import sys
    sys.path.insert(0, "/root/problem")
    import reference
    inputs = {k: np.asarray(v) for k, v in reference.setup_inputs().items()}
    expected = np.asarray(reference.reference(**inputs))
    actual = kernel(**inputs)
    aerr = np.abs(actual - expected)
    denom = np.abs(expected).max()
    print("max abs err:", aerr.max(), "scale:", denom)
    print("rel err:", aerr.max() / denom)

